# revision 4
# baseline (speedup 1.0000x reference)
"""Trainium2 Bass kernel for a 12-layer dense MLP (dims
2-10-20-50-100-200-1000-200-100-50-20-10-2, ReLU after every layer,
softmax over the final 2 logits), data-parallel over 8 NeuronCores.

Layout: feature-major. Activations live in SBUF as [features(partitions),
batch(free)]; weights W[fan_in, fan_out] are used directly as the matmul
stationary operand (lhsT), so each layer is psum[M, F] = W.T @ h[K, F].
Matmuls run in float32r (full-rate fp32 path). ReLU+bias evacuations are
split between ScalarE and VectorE. softmax([a,b]) == [sigmoid(a-b),
sigmoid(b-a)] is computed with one extra 2x2 matmul + Sigmoid.
"""

import numpy as np

import concourse.bass as bass
import concourse.mybir as mybir
import concourse.tile as tile
from concourse import bacc
from concourse.bass_utils import run_bass_kernel_spmd

DIMS = [2, 10, 20, 50, 100, 200, 1000, 200, 100, 50, 20, 10, 2]
N_CORES = 8
N = 262144
B = N // N_CORES  # batch per core
F = 512  # batch columns per matmul (PSUM bank limit for fp32)
T = B // F  # batch tiles per core

F32 = mybir.dt.float32
F32R = mybir.dt.float32r


def _chunks(n: int, maxc: int = 128) -> list[tuple[int, int]]:
    """Split n into near-equal chunks of <=maxc. Returns [(start, size)]."""
    num = -(-n // maxc)
    size = -(-n // num)
    out = []
    s = 0
    while s < n:
        c = min(size, n - s)
        out.append((s, c))
        s += c
    return out


# Static engine assignment for the ReLU+bias evacuation of each
# (layer, m_chunk): True -> ScalarE activation, False -> VectorE.
def _evac_on_act(layer: int, m_idx: int) -> bool:
    if layer == 6:  # 8 chunks: alternate
        return m_idx % 2 == 0
    if layer in (5, 7):  # 2 chunks: split
        return m_idx == 0
    return layer in (2, 9, 10, 12)


def build_nc():
    nc = bacc.Bacc("TRN2", target_bir_lowering=False, debug=False,
                   num_devices=N_CORES)

    # x/weights/D feed f32r matmuls: declare them f32r end-to-end (host
    # pre-rounds the values) so the BIR verifier sees rounded producers.
    x_dram = nc.dram_tensor("xT", [DIMS[0], B], F32R, kind="ExternalInput").ap()
    w_dram = [
        nc.dram_tensor(f"w{l}", [DIMS[l - 1], DIMS[l]], F32R,
                       kind="ExternalInput").ap()
        for l in range(1, len(DIMS))
    ]
    b_dram = [
        nc.dram_tensor(f"b{l}", [DIMS[l], 1], F32, kind="ExternalInput").ap()
        for l in range(1, len(DIMS))
    ]
    d_dram = nc.dram_tensor("D", [2, 2], F32R, kind="ExternalInput").ap()
    y_dram = nc.dram_tensor("y", [2, B], F32, kind="ExternalOutput").ap()

    n_layers = len(DIMS) - 1  # 12

    with tile.TileContext(nc) as tc:
        with (
            tc.tile_pool(name="wpool", bufs=1) as wpool,
            tc.tile_pool(name="hpool", bufs=2) as hpool,
            tc.tile_pool(name="iopool", bufs=4) as iopool,
            tc.tile_pool(name="psum", bufs=8, space="PSUM") as pspool,
        ):
            # ---- load weights/biases once ----
            wt = {}  # (layer, k_idx, m_idx) -> AP
            bt = {}  # (layer, m_idx) -> AP
            for li in range(1, n_layers + 1):
                K, M = DIMS[li - 1], DIMS[li]
                for ki, (ks, kc) in enumerate(_chunks(K)):
                    for mi, (ms, mc) in enumerate(_chunks(M)):
                        w = wpool.tile([kc, mc], F32R, name=f"wt{li}_{ki}_{mi}",
                                       tag=f"wt{li}_{ki}_{mi}", bufs=1)
                        nc.sync.dma_start(
                            w[:], w_dram[li - 1][ks:ks + kc, ms:ms + mc])
                        wt[(li, ki, mi)] = w
                for mi, (ms, mc) in enumerate(_chunks(M)):
                    b = wpool.tile([mc, 1], F32, name=f"bt{li}_{mi}",
                                   tag=f"bt{li}_{mi}", bufs=1)
                    nc.sync.dma_start(b[:], b_dram[li - 1][ms:ms + mc, :])
                    bt[(li, mi)] = b
            d_t = wpool.tile([2, 2], F32R, name="d_t", tag="d_t", bufs=1)
            nc.sync.dma_start(d_t[:], d_dram[:])

            # ---- batch tile loop ----
            for t in range(T):
                xt = iopool.tile([DIMS[0], F], F32R, name=f"xt_{t}", tag="xt",
                                 bufs=4)
                nc.sync.dma_start(xt[:], x_dram[:, t * F:(t + 1) * F])

                h = {0: xt}  # k_idx -> AP for current layer input
                for li in range(1, n_layers + 1):
                    K, M = DIMS[li - 1], DIMS[li]
                    kch = _chunks(K)
                    hnext = {}
                    for mi, (ms, mc) in enumerate(_chunks(M)):
                        ps = pspool.tile([mc, F], F32, name=f"ps{li}_{mi}_{t}",
                                         tag="ps", bufs=8)
                        for ki in range(len(kch)):
                            nc.tensor.matmul(
                                ps[:],
                                wt[(li, ki, mi)][:],
                                h[ki][:],
                                start=(ki == 0),
                                stop=(ki == len(kch) - 1),
                            )
                        hn = hpool.tile([mc, F], F32R, name=f"h{li}_{mi}_{t}",
                                        tag=f"h{li}_{mi}", bufs=2)
                        if _evac_on_act(li, mi):
                            nc.scalar.activation(
                                hn[:], ps[:],
                                mybir.ActivationFunctionType.Relu,
                                bias=bt[(li, mi)][:, 0:1],
                            )
                        else:
                            nc.vector.tensor_scalar(
                                hn[:], ps[:],
                                bt[(li, mi)][:, 0:1], 0.0,
                                mybir.AluOpType.add, mybir.AluOpType.max,
                            )
                        hnext[mi] = hn
                    h = hnext

                # softmax over the 2 logits: [sig(a-b), sig(b-a)]
                psd = pspool.tile([2, F], F32, name=f"psd_{t}", tag="ps",
                                  bufs=8)
                nc.tensor.matmul(psd[:], d_t[:],
                                 h[0][:], start=True, stop=True)
                ot = iopool.tile([2, F], F32, name=f"ot_{t}", tag="ot", bufs=4)
                nc.scalar.activation(ot[:], psd[:],
                                     mybir.ActivationFunctionType.Sigmoid)
                nc.sync.dma_start(y_dram[:, t * F:(t + 1) * F], ot[:])

    nc.compile()
    return nc


_nc_cache = None


def _get_nc():
    global _nc_cache
    if _nc_cache is None:
        _nc_cache = build_nc()
    return _nc_cache


def _round_f32r(a):
    """Round fp32 to f32r (11 mantissa bits), matching the PE's input format."""
    u = np.ascontiguousarray(a, dtype=np.float32).view(np.uint32)
    r = ((u.astype(np.uint64) + 0x800) & 0xFFFFF000).astype(np.uint32)
    return r.view(np.float32)


def _make_in_maps(x, Ws, bs):
    x = _round_f32r(np.asarray(x, dtype=np.float32))
    Ws = [np.ascontiguousarray(_round_f32r(np.asarray(w, dtype=np.float32)))
          for w in Ws]
    bs = [np.ascontiguousarray(np.asarray(b, dtype=np.float32).reshape(-1, 1))
          for b in bs]
    D = np.array([[1.0, -1.0], [-1.0, 1.0]], dtype=np.float32)
    shared = {"D": D}
    for li in range(1, len(DIMS)):
        shared[f"w{li}"] = Ws[li - 1]
        shared[f"b{li}"] = bs[li - 1]
    in_maps = []
    for c in range(N_CORES):
        xT = np.ascontiguousarray(x[c * B:(c + 1) * B].T)
        in_maps.append({"xT": xT, **shared})
    return in_maps


def run(x, Ws, bs, trace=False, **kw):
    nc = _get_nc()
    in_maps = _make_in_maps(x, Ws, bs)
    res = run_bass_kernel_spmd(nc, in_maps, core_ids=list(range(N_CORES)),
                               trace=trace, **kw)
    y = np.concatenate([r["y"].T for r in res.results], axis=0)
    return np.ascontiguousarray(y.astype(np.float32)), res


def kernel(x, Ws, bs):
    y, _ = run(x, Ws, bs, trace=False)
    return y


# revision 5
# speedup vs baseline: 1.2899x; 1.2899x over previous
"""Trainium2 Bass kernel for a 12-layer dense MLP (dims
2-10-20-50-100-200-1000-200-100-50-20-10-2, ReLU after every layer,
softmax over the final 2 logits), data-parallel over 8 NeuronCores.

Layout: feature-major. Activations live in SBUF as [features(partitions),
batch(free)]; weights W[fan_in, fan_out] are the matmul stationary operand
(lhsT), so each layer is psum[M, F] = W.T @ h[K, F]. Matmuls run in bf16
with fp32 PSUM accumulation. Loop order is supertile -> layer -> subtile,
so each layer streams independent 512-col matmul groups back-to-back (no
serial MM->evac->MM stalls on the in-order PE queue), and ReLU+bias
evacuations are wide [mc, 4*512] instructions split between ScalarE and
VectorE. softmax([a,b]) == [sigmoid(a-b), sigmoid(b-a)] is one extra 2x2
matmul + Sigmoid.
"""

import ml_dtypes
import numpy as np

import concourse.bass as bass
import concourse.mybir as mybir
import concourse.tile as tile
from concourse import bacc
from concourse.bass_utils import run_bass_kernel_spmd

DIMS = [2, 10, 20, 50, 100, 200, 1000, 200, 100, 50, 20, 10, 2]
N_CORES = 8
N = 262144
B = N // N_CORES  # batch per core (32768)
F = 512           # batch columns per matmul (PSUM bank, fp32)
SUB = 4           # subtiles per supertile
SW = F * SUB      # supertile width (2048)
ST = B // SW      # supertiles per core (16)

F32 = mybir.dt.float32
BF16 = mybir.dt.bfloat16


def _chunks(n: int, maxc: int = 128) -> list[tuple[int, int]]:
    """Split n into near-equal chunks of <=maxc. Returns [(start, size)]."""
    num = -(-n // maxc)
    size = -(-n // num)
    out = []
    s = 0
    while s < n:
        c = min(size, n - s)
        out.append((s, c))
        s += c
    return out


# Static engine assignment for the ReLU+bias evacuation of each
# (layer, m_chunk): True -> ScalarE activation, False -> VectorE.
def _evac_on_act(layer: int, m_idx: int) -> bool:
    if layer == 6:  # 8 chunks: alternate
        return m_idx % 2 == 0
    if layer in (5, 7):  # 2 chunks: split
        return m_idx == 0
    return layer in (2, 9, 10, 12)


def build_nc():
    nc = bacc.Bacc("TRN2", target_bir_lowering=False, debug=False,
                   num_devices=N_CORES)

    x_dram = nc.dram_tensor("xT", [DIMS[0], B], BF16,
                            kind="ExternalInput").ap()
    w_dram = [
        nc.dram_tensor(f"w{l}", [DIMS[l - 1], DIMS[l]], BF16,
                       kind="ExternalInput").ap()
        for l in range(1, len(DIMS))
    ]
    b_dram = [
        nc.dram_tensor(f"b{l}", [DIMS[l], 1], F32, kind="ExternalInput").ap()
        for l in range(1, len(DIMS))
    ]
    d_dram = nc.dram_tensor("D", [2, 2], BF16, kind="ExternalInput").ap()
    y_dram = nc.dram_tensor("y", [2, B], F32, kind="ExternalOutput").ap()

    n_layers = len(DIMS) - 1  # 12

    with tile.TileContext(nc) as tc:
        with (
            tc.tile_pool(name="wpool", bufs=1) as wpool,
            tc.tile_pool(name="hpool", bufs=1) as hpool,
            tc.tile_pool(name="iopool", bufs=2) as iopool,
            tc.tile_pool(name="psum", bufs=2, space="PSUM") as pspool,
        ):
            # ---- load weights/biases once ----
            wt = {}
            bt = {}
            for li in range(1, n_layers + 1):
                K, M = DIMS[li - 1], DIMS[li]
                for ki, (ks, kc) in enumerate(_chunks(K)):
                    for mi, (ms, mc) in enumerate(_chunks(M)):
                        w = wpool.tile([kc, mc], BF16,
                                       name=f"wt{li}_{ki}_{mi}",
                                       tag=f"wt{li}_{ki}_{mi}", bufs=1)
                        nc.sync.dma_start(
                            w[:], w_dram[li - 1][ks:ks + kc, ms:ms + mc])
                        wt[(li, ki, mi)] = w
                for mi, (ms, mc) in enumerate(_chunks(M)):
                    b = wpool.tile([mc, 1], F32, name=f"bt{li}_{mi}",
                                   tag=f"bt{li}_{mi}", bufs=1)
                    nc.sync.dma_start(b[:], b_dram[li - 1][ms:ms + mc, :])
                    bt[(li, mi)] = b
            d_t = wpool.tile([2, 2], BF16, name="d_t", tag="d_t", bufs=1)
            nc.sync.dma_start(d_t[:], d_dram[:])

            # ---- supertile loop ----
            for s in range(ST):
                xt = iopool.tile([DIMS[0], SW], BF16, name=f"xt_{s}",
                                 tag="xt", bufs=2)
                nc.sync.dma_start(xt[:], x_dram[:, s * SW:(s + 1) * SW])

                h = {0: xt}
                for li in range(1, n_layers + 1):
                    K, M = DIMS[li - 1], DIMS[li]
                    kch = _chunks(K)
                    hnext = {}
                    for mi, (ms, mc) in enumerate(_chunks(M)):
                        ps = pspool.tile([mc, SW], F32,
                                         name=f"ps{li}_{mi}_{s}", tag="ps",
                                         bufs=2)
                        for ki in range(len(kch)):
                            for f in range(SUB):
                                nc.tensor.matmul(
                                    ps[:, f * F:(f + 1) * F],
                                    wt[(li, ki, mi)][:],
                                    h[ki][:, f * F:(f + 1) * F],
                                    start=(ki == 0),
                                    stop=(ki == len(kch) - 1),
                                )
                        hn = hpool.tile([mc, SW], BF16,
                                        name=f"h{li}_{mi}_{s}",
                                        tag=f"h{li}_{mi}", bufs=1)
                        if _evac_on_act(li, mi):
                            nc.scalar.activation(
                                hn[:], ps[:],
                                mybir.ActivationFunctionType.Relu,
                                bias=bt[(li, mi)][:, 0:1],
                            )
                        else:
                            nc.vector.tensor_scalar(
                                hn[:], ps[:],
                                bt[(li, mi)][:, 0:1], 0.0,
                                mybir.AluOpType.add, mybir.AluOpType.max,
                            )
                        hnext[mi] = hn
                    h = hnext

                # softmax over the 2 logits: [sig(a-b), sig(b-a)]
                psd = pspool.tile([2, SW], F32, name=f"psd_{s}", tag="ps",
                                  bufs=2)
                for f in range(SUB):
                    nc.tensor.matmul(psd[:, f * F:(f + 1) * F], d_t[:],
                                     h[0][:, f * F:(f + 1) * F],
                                     start=True, stop=True)
                ot = iopool.tile([2, SW], F32, name=f"ot_{s}", tag="ot",
                                 bufs=2)
                nc.scalar.activation(ot[:], psd[:],
                                     mybir.ActivationFunctionType.Sigmoid)
                nc.sync.dma_start(y_dram[:, s * SW:(s + 1) * SW], ot[:])

    nc.compile()
    return nc


_nc_cache = None


def _get_nc():
    global _nc_cache
    if _nc_cache is None:
        _nc_cache = build_nc()
    return _nc_cache


def _make_in_maps(x, Ws, bs):
    x = np.asarray(x, dtype=np.float32)
    Ws = [np.ascontiguousarray(
        np.asarray(w, dtype=np.float32).astype(ml_dtypes.bfloat16))
        for w in Ws]
    bs = [np.ascontiguousarray(np.asarray(b, dtype=np.float32).reshape(-1, 1))
          for b in bs]
    D = np.array([[1.0, -1.0], [-1.0, 1.0]], dtype=ml_dtypes.bfloat16)
    shared = {"D": D}
    for li in range(1, len(DIMS)):
        shared[f"w{li}"] = Ws[li - 1]
        shared[f"b{li}"] = bs[li - 1]
    in_maps = []
    for c in range(N_CORES):
        xT = np.ascontiguousarray(x[c * B:(c + 1) * B].T
                                  .astype(ml_dtypes.bfloat16))
        in_maps.append({"xT": xT, **shared})
    return in_maps


def run(x, Ws, bs, trace=False, **kw):
    nc = _get_nc()
    in_maps = _make_in_maps(x, Ws, bs)
    res = run_bass_kernel_spmd(nc, in_maps, core_ids=list(range(N_CORES)),
                               trace=trace, **kw)
    y = np.concatenate([r["y"].T for r in res.results], axis=0)
    return np.ascontiguousarray(y.astype(np.float32)), res


def kernel(x, Ws, bs):
    y, _ = run(x, Ws, bs, trace=False)
    return y


# revision 8
# speedup vs baseline: 2.0998x; 1.6278x over previous
"""Trainium2 Bass kernel for a 12-layer dense MLP (dims
2-10-20-50-100-200-1000-200-100-50-20-10-2, ReLU after every layer,
softmax over the final 2 logits), data-parallel over 8 NeuronCores.

Layout: feature-major. Activations live in SBUF as [features(partitions),
batch(free)]; weights W[fan_in, fan_out] are the matmul stationary operand
(lhsT), so each layer is psum[M, F] = W.T @ h[K, F]. Matmuls run in bf16
with fp32 PSUM accumulation.

Loop order is supertile (4096 cols) -> layer -> evac-group (1024 cols) ->
k-chunk -> 512-col block, so each layer streams independent matmuls
back-to-back and the in-order PE queue never stalls on the
matmul->ReLU->matmul chain. Small layers' activations are packed into
shared 128-partition tiles at 32-aligned bases (tile_position routes the
matmuls to matching PE array tiles). ReLU+bias evacuations are [mc, 1024]
instructions balanced between ScalarE and VectorE.

softmax([a,b]) == [sigmoid(a-b), sigmoid(b-a)] is one extra 2x2 matmul +
Sigmoid per group.
"""

import ml_dtypes
import numpy as np

import concourse.bass as bass
import concourse.mybir as mybir
import concourse.tile as tile
from concourse import bacc
from concourse.bass_utils import run_bass_kernel_spmd

DIMS = [2, 10, 20, 50, 100, 200, 1000, 200, 100, 50, 20, 10, 2]
N_CORES = 8
N = 262144
B = N // N_CORES   # batch per core (32768)
F = 512            # columns per matmul (PSUM bank, fp32)
SUB = 8            # 512-col blocks per supertile
SW = F * SUB       # supertile width (4096)
ST = B // SW       # supertiles per core (8)
EG = 1024          # evac group width
GPB = EG // F      # blocks per evac group (2)
G = SW // EG       # evac groups per supertile (4)

F32 = mybir.dt.float32
BF16 = mybir.dt.bfloat16

N_LAYERS = len(DIMS) - 1  # 12


def _chunks(n: int, maxc: int = 128) -> list[tuple[int, int]]:
    num = -(-n // maxc)
    size = -(-n // num)
    out = []
    s = 0
    while s < n:
        c = min(size, n - s)
        out.append((s, c))
        s += c
    return out


# h placement: htensor name, partition base inside it, for each layer's
# output. Lifetime-disjoint layers share a tensor tag. X1 packs h1/h2/h3
# (+h12), X2 packs h9/h10/h11. h4/h8 share "h100"; h5/h7 share "h200_*".
H_PLACE = {
    1: [("X1", 0)],
    2: [("X1", 32)],
    3: [("X1", 64)],
    4: [("h100", 0)],
    5: [("h200_0", 0), ("h200_1", 0)],
    6: [(f"h6_{m}", 0) for m in range(8)],
    7: [("h200_0", 0), ("h200_1", 0)],
    8: [("h100", 0)],
    9: [("X2", 0)],
    10: [("X2", 64)],
    11: [("X2", 96)],
    12: [("X1", 0)],
}
# htensor name -> partition rows
H_SIZE = {"X1": 128, "X2": 128, "h100": 100, "h200_0": 100, "h200_1": 100,
          **{f"h6_{m}": 125 for m in range(8)}}


def build_nc():
    nc = bacc.Bacc("TRN2", target_bir_lowering=False, debug=False,
                   num_devices=N_CORES)

    x_dram = nc.dram_tensor("xT", [DIMS[0], B], BF16,
                            kind="ExternalInput").ap()
    w_dram = [
        nc.dram_tensor(f"w{l}", [DIMS[l - 1], DIMS[l]], BF16,
                       kind="ExternalInput").ap()
        for l in range(1, N_LAYERS + 1)
    ]
    b_dram = [
        nc.dram_tensor(f"b{l}", [DIMS[l], 1], F32, kind="ExternalInput").ap()
        for l in range(1, N_LAYERS + 1)
    ]
    d_dram = nc.dram_tensor("D", [2, 2], BF16, kind="ExternalInput").ap()
    y_dram = nc.dram_tensor("y", [2, B], F32, kind="ExternalOutput").ap()

    # engine balance state (ns of work assigned)
    eng_load = {"act": 0.0, "dve": 0.0}

    def evac(out_ap, in_ap, bias_ap, pin_act=False):
        act_cost = (EG + 310) / 1.2
        dve_cost = (EG + 205) / 0.96
        use_act = pin_act or (eng_load["act"] + act_cost
                              <= eng_load["dve"] + dve_cost)
        if use_act:
            eng_load["act"] += act_cost
            nc.scalar.activation(out_ap, in_ap,
                                 mybir.ActivationFunctionType.Relu,
                                 bias=bias_ap)
        else:
            eng_load["dve"] += dve_cost
            nc.vector.tensor_scalar(out_ap, in_ap, bias_ap, 0.0,
                                    mybir.AluOpType.add, mybir.AluOpType.max)

    with tile.TileContext(nc) as tc:
        with (
            tc.tile_pool(name="wpool", bufs=1) as wpool,
            tc.tile_pool(name="hpool", bufs=1) as hpool,
            tc.tile_pool(name="iopool", bufs=2) as iopool,
            tc.tile_pool(name="psum", bufs=4, space="PSUM") as pspool,
        ):
            # ---- load weights/biases once, placed at their row bases ----
            wt = {}   # (layer, k_idx, m_idx) -> AP sliced at row base
            bt = {}   # (layer, m_idx) -> bias AP sliced at col base
            rbase = {}  # row base for layer l's rhs (= place of h_{l-1})
            for li in range(1, N_LAYERS + 1):
                rbase[li] = 0 if li == 1 else H_PLACE[li - 1][0][1]
            # multi-chunk rhs layers (5->6, 6->7, 7->8) all have base 0
            for li in range(1, N_LAYERS + 1):
                K, M = DIMS[li - 1], DIMS[li]
                rb = rbase[li]
                for ki, (ks, kc) in enumerate(_chunks(K)):
                    krb = rb if len(_chunks(K)) == 1 else 0
                    for mi, (ms, mc) in enumerate(_chunks(M)):
                        w = wpool.tile([krb + kc, mc], BF16,
                                       name=f"wt{li}_{ki}_{mi}",
                                       tag=f"wt{li}_{ki}_{mi}", bufs=1)
                        nc.sync.dma_start(
                            w[krb:krb + kc, :],
                            w_dram[li - 1][ks:ks + kc, ms:ms + mc])
                        wt[(li, ki, mi)] = w[krb:krb + kc, :]
                for mi, (ms, mc) in enumerate(_chunks(M)):
                    cb = H_PLACE[li][mi][1]
                    b = wpool.tile([cb + mc, 1], F32, name=f"bt{li}_{mi}",
                                   tag=f"bt{li}_{mi}", bufs=1)
                    nc.sync.dma_start(b[cb:cb + mc, :],
                                      b_dram[li - 1][ms:ms + mc, :])
                    bt[(li, mi)] = b[cb:cb + mc, 0:1]
            d_t = wpool.tile([2, 2], BF16, name="d_t", tag="d_t", bufs=1)
            nc.sync.dma_start(d_t[:], d_dram[:])

            # ---- supertile loop ----
            for s in range(ST):
                xt = iopool.tile([DIMS[0], SW], BF16, name=f"xt_{s}",
                                 tag="xt", bufs=2)
                nc.sync.dma_start(xt[:], x_dram[:, s * SW:(s + 1) * SW])

                # h tensors for this supertile, allocated lazily per tag
                htiles = {}

                def htile(tag):
                    if tag not in htiles:
                        htiles[tag] = hpool.tile(
                            [H_SIZE[tag], SW], BF16, name=f"{tag}_{s}",
                            tag=tag, bufs=1)
                    return htiles[tag]

                # rhs chunks of the current layer input:
                # list of (ap_full_width, row_base)
                hin = [(xt, 0)]
                for li in range(1, N_LAYERS + 1):
                    K, M = DIMS[li - 1], DIMS[li]
                    kch = _chunks(K)
                    mch = _chunks(M)
                    single_m = len(mch) == 1
                    for g in range(G):
                        for mi, (ms, mc) in enumerate(mch):
                            cb = H_PLACE[li][mi][1]
                            ps = pspool.tile([128, EG], F32,
                                             name=f"ps{li}_{mi}_{g}_{s}",
                                             tag="ps", bufs=4)
                            for ki, (ks, kc) in enumerate(kch):
                                rhs, rb = hin[ki]
                                for f in range(GPB):
                                    c0 = g * EG + f * F
                                    nc.tensor.matmul(
                                        ps[cb:cb + mc, f * F:(f + 1) * F],
                                        wt[(li, ki, mi)],
                                        rhs[rb:rb + kc, c0:c0 + F],
                                        start=(ki == 0),
                                        stop=(ki == len(kch) - 1),
                                        tile_position=(rb, cb),
                                    )
                            ht = htile(H_PLACE[li][mi][0])
                            evac(ht[cb:cb + mc, g * EG:(g + 1) * EG],
                                 ps[cb:cb + mc, :],
                                 bt[(li, mi)],
                                 pin_act=(single_m and g == 0))
                    # next layer's input chunks
                    if len(mch) == 1:
                        tag, cb = H_PLACE[li][0]
                        hin = [(htile(tag), cb)]
                    else:
                        hin = [(htile(H_PLACE[li][mi][0]), H_PLACE[li][mi][1])
                               for mi in range(len(mch))]

                # softmax over the 2 logits: [sig(a-b), sig(b-a)]
                ot = iopool.tile([2, SW], F32, name=f"ot_{s}", tag="ot",
                                 bufs=2)
                h12, rb12 = hin[0]
                for g in range(G):
                    psd = pspool.tile([128, EG], F32, name=f"psd_{g}_{s}",
                                      tag="ps", bufs=4)
                    for f in range(GPB):
                        c0 = g * EG + f * F
                        nc.tensor.matmul(
                            psd[0:2, f * F:(f + 1) * F], d_t[:],
                            h12[rb12:rb12 + 2, c0:c0 + F],
                            start=True, stop=True, tile_position=(rb12, 0))
                    nc.scalar.activation(
                        ot[:, g * EG:(g + 1) * EG], psd[0:2, :],
                        mybir.ActivationFunctionType.Sigmoid)
                nc.sync.dma_start(y_dram[:, s * SW:(s + 1) * SW], ot[:])

    nc.compile()
    return nc


_nc_cache = None


def _get_nc():
    global _nc_cache
    if _nc_cache is None:
        _nc_cache = build_nc()
    return _nc_cache


def _make_in_maps(x, Ws, bs):
    x = np.asarray(x, dtype=np.float32)
    Ws = [np.ascontiguousarray(
        np.asarray(w, dtype=np.float32).astype(ml_dtypes.bfloat16))
        for w in Ws]
    bs = [np.ascontiguousarray(np.asarray(b, dtype=np.float32).reshape(-1, 1))
          for b in bs]
    D = np.array([[1.0, -1.0], [-1.0, 1.0]], dtype=ml_dtypes.bfloat16)
    shared = {"D": D}
    for li in range(1, len(DIMS)):
        shared[f"w{li}"] = Ws[li - 1]
        shared[f"b{li}"] = bs[li - 1]
    in_maps = []
    for c in range(N_CORES):
        xT = np.ascontiguousarray(x[c * B:(c + 1) * B].T
                                  .astype(ml_dtypes.bfloat16))
        in_maps.append({"xT": xT, **shared})
    return in_maps


def run(x, Ws, bs, trace=False, **kw):
    nc = _get_nc()
    in_maps = _make_in_maps(x, Ws, bs)
    res = run_bass_kernel_spmd(nc, in_maps, core_ids=list(range(N_CORES)),
                               trace=trace, **kw)
    y = np.concatenate([r["y"].T for r in res.results], axis=0)
    return np.ascontiguousarray(y.astype(np.float32)), res


def kernel(x, Ws, bs):
    y, _ = run(x, Ws, bs, trace=False)
    return y


# revision 11
# speedup vs baseline: 2.2374x; 1.0655x over previous
"""Trainium2 Bass kernel for a 12-layer dense MLP (dims
2-10-20-50-100-200-1000-200-100-50-20-10-2, ReLU after every layer,
softmax over the final 2 logits), data-parallel over 8 NeuronCores.

Layout: feature-major. Activations live in SBUF as [features(partitions),
batch(free)]; weights W[fan_in, fan_out] are the matmul stationary operand
(lhsT), so each layer is psum[M, F] = W.T @ h[K, F]. Matmuls run in bf16
with fp32 PSUM accumulation.

Loop order is supertile (4096 cols) -> layer -> evac-group (1024 cols) ->
k-chunk -> 512-col block, so each layer streams independent matmuls
back-to-back and the in-order PE queue never stalls on the
matmul->ReLU->matmul chain. Small layers' activations are packed into
shared 128-partition tiles at 32-aligned bases (tile_position routes the
matmuls to matching PE array tiles). ReLU+bias evacuations are [mc, 1024]
instructions balanced between ScalarE and VectorE.

softmax([a,b]) == [sigmoid(a-b), sigmoid(b-a)] is one extra 2x2 matmul +
Sigmoid per group.
"""

import ml_dtypes
import numpy as np

import concourse.bass as bass
import concourse.mybir as mybir
import concourse.tile as tile
from concourse import bacc
from concourse.bass_utils import run_bass_kernel_spmd

DIMS = [2, 10, 20, 50, 100, 200, 1000, 200, 100, 50, 20, 10, 2]
N_CORES = 8
N = 262144
B = N // N_CORES   # batch per core (32768)
F = 512            # columns per matmul (PSUM bank, fp32)
SUB = 8            # 512-col blocks per supertile
SW = F * SUB       # supertile width (4096)
ST = B // SW       # supertiles per core (8)
EG = 1024          # evac group width
GPB = EG // F      # blocks per evac group (2)
G = SW // EG       # evac groups per supertile (4)

F32 = mybir.dt.float32
BF16 = mybir.dt.bfloat16

N_LAYERS = len(DIMS) - 1  # 12


def _chunks(n: int, maxc: int = 128) -> list[tuple[int, int]]:
    num = -(-n // maxc)
    size = -(-n // num)
    out = []
    s = 0
    while s < n:
        c = min(size, n - s)
        out.append((s, c))
        s += c
    return out


# h placement: htensor name, partition base inside it, for each layer's
# output. Lifetime-disjoint layers share a tensor tag. X1 packs h1/h2/h3
# (+h12), X2 packs h9/h10/h11. h4/h8 share "h100"; h5/h7 share "h200_*".
H_PLACE = {
    1: [("X1", 0)],
    2: [("X1", 32)],
    3: [("X1", 64)],
    4: [("h100", 0)],
    5: [("h200_0", 0), ("h200_1", 0)],
    6: [(f"h6_{m}", 0) for m in range(8)],
    7: [("h200_0", 0), ("h200_1", 0)],
    8: [("h100", 0)],
    9: [("X2", 0)],
    10: [("X2", 64)],
    11: [("X2", 96)],
    12: [("X1", 0)],
}
# htensor name -> partition rows
H_SIZE = {"X1": 128, "X2": 128, "h100": 100, "h200_0": 100, "h200_1": 100,
          **{f"h6_{m}": 125 for m in range(8)}}


def build_nc():
    nc = bacc.Bacc("TRN2", target_bir_lowering=False, debug=False,
                   num_devices=N_CORES)

    x_dram = nc.dram_tensor("xT", [DIMS[0], B], BF16,
                            kind="ExternalInput").ap()
    w_dram = [
        nc.dram_tensor(f"w{l}", [DIMS[l - 1], DIMS[l]], BF16,
                       kind="ExternalInput").ap()
        for l in range(1, N_LAYERS + 1)
    ]
    b_dram = [
        nc.dram_tensor(f"b{l}", [DIMS[l], 1], F32, kind="ExternalInput").ap()
        for l in range(1, N_LAYERS + 1)
    ]
    d_dram = nc.dram_tensor("D", [2, 2], BF16, kind="ExternalInput").ap()
    y_dram = nc.dram_tensor("y", [2, B], F32, kind="ExternalOutput").ap()

    # engine balance state (ns of work assigned)
    eng_load = {"act": 0.0, "dve": 0.0}

    def evac(out_ap, in_ap, bias_ap, pin_act=False):
        act_cost = (EG + 310) / 1.2
        dve_cost = (EG + 205) / 0.96
        use_act = pin_act or (eng_load["act"] + act_cost
                              <= eng_load["dve"] + dve_cost)
        if use_act:
            eng_load["act"] += act_cost
            nc.scalar.activation(out_ap, in_ap,
                                 mybir.ActivationFunctionType.Relu,
                                 bias=bias_ap)
        else:
            eng_load["dve"] += dve_cost
            nc.vector.tensor_scalar(out_ap, in_ap, bias_ap, 0.0,
                                    mybir.AluOpType.add, mybir.AluOpType.max)

    with tile.TileContext(nc) as tc:
        with (
            tc.tile_pool(name="wpool", bufs=1) as wpool,
            tc.tile_pool(name="hpool", bufs=1) as hpool,
            tc.tile_pool(name="iopool", bufs=2) as iopool,
            tc.tile_pool(name="psum", bufs=4, space="PSUM") as pspool,
        ):
            # ---- load weights/biases once, placed at their row bases ----
            wt = {}   # (layer, k_idx, m_idx) -> AP sliced at row base
            bt = {}   # (layer, m_idx) -> bias AP sliced at col base
            rbase = {}  # row base for layer l's rhs (= place of h_{l-1})
            for li in range(1, N_LAYERS + 1):
                rbase[li] = 0 if li == 1 else H_PLACE[li - 1][0][1]
            # multi-chunk rhs layers (5->6, 6->7, 7->8) all have base 0
            for li in range(1, N_LAYERS + 1):
                K, M = DIMS[li - 1], DIMS[li]
                rb = rbase[li]
                for ki, (ks, kc) in enumerate(_chunks(K)):
                    krb = rb if len(_chunks(K)) == 1 else 0
                    for mi, (ms, mc) in enumerate(_chunks(M)):
                        w = wpool.tile([krb + kc, mc], BF16,
                                       name=f"wt{li}_{ki}_{mi}",
                                       tag=f"wt{li}_{ki}_{mi}", bufs=1)
                        nc.sync.dma_start(
                            w[krb:krb + kc, :],
                            w_dram[li - 1][ks:ks + kc, ms:ms + mc])
                        wt[(li, ki, mi)] = w[krb:krb + kc, :]
                for mi, (ms, mc) in enumerate(_chunks(M)):
                    cb = H_PLACE[li][mi][1]
                    b = wpool.tile([cb + mc, 1], F32, name=f"bt{li}_{mi}",
                                   tag=f"bt{li}_{mi}", bufs=1)
                    nc.sync.dma_start(b[cb:cb + mc, :],
                                      b_dram[li - 1][ms:ms + mc, :])
                    bt[(li, mi)] = b[cb:cb + mc, 0:1]
            d_t = wpool.tile([2, 2], BF16, name="d_t", tag="d_t", bufs=1)
            nc.sync.dma_start(d_t[:], d_dram[:])
            # ---- supertile loop ----
            for s in range(ST):
                xt = iopool.tile([DIMS[0], SW], BF16, name=f"xt_{s}",
                                 tag="xt", bufs=2)
                nc.gpsimd.dma_start(xt[:], x_dram[:, s * SW:(s + 1) * SW])

                # h tensors for this supertile, allocated lazily per tag
                htiles = {}

                def htile(tag):
                    if tag not in htiles:
                        htiles[tag] = hpool.tile(
                            [H_SIZE[tag], SW], BF16, name=f"{tag}_{s}",
                            tag=tag, bufs=1)
                    return htiles[tag]

                # rhs chunks of the current layer input:
                # list of (ap_full_width, row_base)
                hin = [(xt, 0)]
                for li in range(1, N_LAYERS + 1):
                    K, M = DIMS[li - 1], DIMS[li]
                    kch = _chunks(K)
                    mch = _chunks(M)
                    single_m = len(mch) == 1
                    for g in range(G):
                        for mi, (ms, mc) in enumerate(mch):
                            cb = H_PLACE[li][mi][1]
                            ps = pspool.tile([128, EG], F32,
                                             name=f"ps{li}_{mi}_{g}_{s}",
                                             tag="ps", bufs=4)
                            for ki, (ks, kc) in enumerate(kch):
                                rhs, rb = hin[ki]
                                for f in range(GPB):
                                    c0 = g * EG + f * F
                                    nc.tensor.matmul(
                                        ps[cb:cb + mc, f * F:(f + 1) * F],
                                        wt[(li, ki, mi)],
                                        rhs[rb:rb + kc, c0:c0 + F],
                                        start=(ki == 0),
                                        stop=(ki == len(kch) - 1),
                                        tile_position=(rb, cb),
                                    )
                            ht = htile(H_PLACE[li][mi][0])
                            evac(ht[cb:cb + mc, g * EG:(g + 1) * EG],
                                 ps[cb:cb + mc, :],
                                 bt[(li, mi)],
                                 pin_act=(single_m and g == 0))
                    # next layer's input chunks
                    if len(mch) == 1:
                        tag, cb = H_PLACE[li][0]
                        hin = [(htile(tag), cb)]
                    else:
                        hin = [(htile(H_PLACE[li][mi][0]), H_PLACE[li][mi][1])
                               for mi in range(len(mch))]

                # softmax over the 2 logits: [sig(a-b), sig(b-a)] via a
                # 2x2 difference matmul + Sigmoid
                ot = iopool.tile([2, SW], F32, name=f"ot_{s}", tag="ot",
                                 bufs=2)
                h12, rb12 = hin[0]
                for g in range(G):
                    psd = pspool.tile([128, EG], F32, name=f"psd_{g}_{s}",
                                      tag="ps", bufs=4)
                    for f in range(GPB):
                        c0 = g * EG + f * F
                        nc.tensor.matmul(
                            psd[0:2, f * F:(f + 1) * F], d_t[:],
                            h12[rb12:rb12 + 2, c0:c0 + F],
                            start=True, stop=True, tile_position=(rb12, 0))
                    nc.scalar.activation(
                        ot[:, g * EG:(g + 1) * EG], psd[0:2, :],
                        mybir.ActivationFunctionType.Sigmoid)
                nc.gpsimd.dma_start(y_dram[:, s * SW:(s + 1) * SW], ot[:])

    nc.compile()
    return nc


_nc_cache = None


def _get_nc():
    global _nc_cache
    if _nc_cache is None:
        _nc_cache = build_nc()
    return _nc_cache


def _make_in_maps(x, Ws, bs):
    x = np.asarray(x, dtype=np.float32)
    Ws = [np.ascontiguousarray(
        np.asarray(w, dtype=np.float32).astype(ml_dtypes.bfloat16))
        for w in Ws]
    bs = [np.ascontiguousarray(np.asarray(b, dtype=np.float32).reshape(-1, 1))
          for b in bs]
    D = np.array([[1.0, -1.0], [-1.0, 1.0]], dtype=ml_dtypes.bfloat16)
    shared = {"D": D}
    for li in range(1, len(DIMS)):
        shared[f"w{li}"] = Ws[li - 1]
        shared[f"b{li}"] = bs[li - 1]
    in_maps = []
    for c in range(N_CORES):
        xT = np.ascontiguousarray(x[c * B:(c + 1) * B].T
                                  .astype(ml_dtypes.bfloat16))
        in_maps.append({"xT": xT, **shared})
    return in_maps


def run(x, Ws, bs, trace=False, **kw):
    nc = _get_nc()
    in_maps = _make_in_maps(x, Ws, bs)
    res = run_bass_kernel_spmd(nc, in_maps, core_ids=list(range(N_CORES)),
                               trace=trace, **kw)
    y = np.concatenate([r["y"].T for r in res.results], axis=0)
    return np.ascontiguousarray(y.astype(np.float32)), res


def kernel(x, Ws, bs):
    y, _ = run(x, Ws, bs, trace=False)
    return y
